# revision 7
# baseline (speedup 1.0000x reference)
"""GAT layer (N=8192, D=64) as a Bass/Tile kernel on 8 TRN2 NeuronCores.

Math (reference):
    h  = x @ W.T + b
    s1 = h @ a1 ; s2 = h @ a2                    # [N] each
    score[i,j] = s2[i] + s1[j]
    att = softmax_j(leaky_relu(score))
    out = att @ x

Reformulation:
    Fold the linear layer:  v = W.T @ [a1|a2], c_k = b.a_k
      p1 = x @ v1 ; p2 = x @ v2 ; s1 = p1 + c1 ; s2 = p2 + c2
    Softmax rows are shift invariant, so subtract p2[i] from row i. With
    per-j scalars E1 = exp(sh1), F1 = exp(0.01*sh1) (sh1 = p1 + c1 + c2)
    and the broadcast tile G2b[j,i] = exp(-0.99*p2[i]):
      e[j,i] = max( G2b[j,i] * F1[j],  E1[j] )
    The final matmul (ones-column appended to x for the softmax
    denominator) accumulates over j in PSUM:
      outT[0:65, i] += x_ext[j,:].T @ e[j,i] ; Z[i] = outT[64,i]

Main optimization vs the previous version: e-tiles are produced on THREE
engines concurrently so the PE never starves and can ramp to its max
p-state (2.4 GHz), which halves matmul cost:
  - DVE:  e = max(G2b*F1, E1)          (tensor_scalar, 2x bf16 mode)
  - ACT:  r = relu(E1 - F1*G2b)        (activation Relu, AP scale/bias)
          using max(A,B) = A + relu(B-A); the missing rank-1 part
          A = F1[j]*G2[i] is restored by a single K=1 matmul at the end:
          acc += TF_act[d] * G2row[i],  TF_act[d] = sum_{j in ACT} x~[j,d]F1[j]
          (TF_act accumulates via one 1-column matmul per ACT tile).
  - POOL: same op as DVE (gpsimd tensor_scalar).
The s1 projection runs on PE as 64 tiny quadrant matmuls against xT
(fp32r), with E1/F1 exps reading the PSUM result directly on ACT, so
DVE/POOL/ACT stay dedicated to e-tile production.

Sharding: each core owns N/8 = 1024 query rows i. The host rotates the
j-order per core so the core's own block is always j 0..1023 (all cores
run the identical program; softmax sums are order-invariant).
"""

import sys
import types

import ml_dtypes
import numpy as np

import concourse.bacc as bacc
import concourse.bass as bass
import concourse.mybir as mybir
import concourse.tile as tile
from concourse.bass_utils import run_bass_kernel_spmd


def _install_ntff_hook_shim():
    """The agent image's ``antenv`` lacks ``axon_hooks``; provide it so
    ``run_bass_kernel_spmd(trace=True)`` can capture NTFF profiles."""
    if "antenv.axon_hooks" in sys.modules:
        return
    try:
        from trn_agent_boot.trn_boot import _ntff_profile_via_ctypes

        hook = _ntff_profile_via_ctypes("/opt/axon/libaxon_pjrt.so")
        mod = types.ModuleType("antenv.axon_hooks")
        mod._hook = hook
        mod.get_axon_ntff_profile_hook = lambda: mod._hook
        mod.set_axon_ntff_profile_hook = lambda h: setattr(mod, "_hook", h)
        sys.modules["antenv.axon_hooks"] = mod
    except Exception:
        pass


_install_ntff_hook_shim()

N, D = 8192, 64
NCORES = 8
RB = N // NCORES          # rows (i) per core = 1024
NT = N // 128             # j tiles of 128 = 64
BT = RB // 128            # i tiles per core = 8
DE = D + 1                # x extended with ones column = 65
F32 = mybir.dt.float32
F32R = mybir.dt.float32r
BF16 = mybir.dt.bfloat16
EXP = mybir.ActivationFunctionType.Exp
RELU = mybir.ActivationFunctionType.Relu
ADD = mybir.AluOpType.add
MUL = mybir.AluOpType.mult
MAX = mybir.AluOpType.max
AX_X = mybir.AxisListType.X
PKW = D + 3 + 128  # packed small-input width (W | b | a1 | a2 | ident)

# j-tile processing order: follow the xq DMA slabs (slab k covers lower
# tiles 8k..8k+7 and upper tiles 32+8k..32+8k+7).
CHUNKS = [0, 4, 1, 5, 2, 6, 3, 7]        # s1 chunk order (chunk c = tiles 8c..8c+7)
JORDER = [t for c in CHUNKS for t in range(8 * c, 8 * c + 8)]
# engine per position in JORDER: DVE 32, ACT 16, POOL 16
EPAT = ["V", "A", "V", "P"]


def build_bass() -> bass.Bass:
    nc = bacc.Bacc(None)
    # xq: x.T quadrant-packed, fp32: partitions 0:64 = d, cols j 0..4095;
    # partitions 64:128 = d, cols j 4096..8191 (j in the per-core rotated
    # order; own block is always j 0..1023).
    xq_d = nc.declare_dram_parameter("xq", [128, 4096], F32R, isOutput=False)
    # xbf: bf16 (x | ones) partition-major: [128, t, de]
    xbf_d = nc.declare_dram_parameter("xbf", [128, NT * DE], BF16, isOutput=False)
    pk_d = nc.declare_dram_parameter("pack", [128, PKW], F32, isOutput=False)
    out_d = nc.declare_dram_parameter("out", [128, BT * D], F32, isOutput=True)

    with tile.TileContext(nc) as tc:
        with (
            tc.tile_pool(name="persist", bufs=1) as persist,
            tc.tile_pool(name="small", bufs=1) as small,
            tc.tile_pool(name="epool", bufs=10) as epool,
            tc.tile_pool(name="opool", bufs=2) as opool,
            tc.tile_pool(name="psumA", bufs=3, space="PSUM") as psumA,
            tc.tile_pool(name="psumS", bufs=2, space="PSUM") as psumS,
            tc.tile_pool(name="psumB", bufs=1, space="PSUM") as psumB,
        ):
            # ---------------- DMAs (SP queue, ordered) ----------------
            pk = small.tile([128, PKW], F32)
            nc.sync.dma_start(pk, pk_d[:, :])
            W_lo = pk[0:D, 0:D]
            b_lo = pk[0:D, D : D + 1]
            a_lo = pk[0:D, D + 1 : D + 3]
            W_hi = pk[64:128, 0:D]
            a_hi = pk[64:128, D + 1 : D + 3]
            ident = pk[:, D + 3 : D + 3 + 128]

            xq = persist.tile([128, 4096], F32R)
            xbf_flat = persist.tile([128, NT * DE], BF16)
            x_bf = xbf_flat.rearrange("p (t d) -> p t d", t=NT)
            # slab k of xq covers j-tiles 8k..8k+7 (lower) and 32+8k..39+8k
            # (upper); xbf slab m covers tiles 16m..16m+15.  Issue in the
            # order the pipeline consumes them.
            nc.sync.dma_start(xq[:, 0:1024], xq_d[:, 0:1024])
            nc.sync.dma_start(xbf_flat[:, 0 : 16 * DE], xbf_d[:, 0 : 16 * DE])
            nc.sync.dma_start(
                xbf_flat[:, 32 * DE : 48 * DE], xbf_d[:, 32 * DE : 48 * DE]
            )
            nc.sync.dma_start(xq[:, 1024:2048], xq_d[:, 1024:2048])
            nc.sync.dma_start(xbf_flat[:, 16 * DE : 32 * DE], xbf_d[:, 16 * DE : 32 * DE])
            nc.sync.dma_start(xq[:, 2048:3072], xq_d[:, 2048:3072])
            nc.sync.dma_start(
                xbf_flat[:, 48 * DE : 64 * DE], xbf_d[:, 48 * DE : 64 * DE]
            )
            nc.sync.dma_start(xq[:, 3072:4096], xq_d[:, 3072:4096])

            ones_row = small.tile([1, 128], F32)
            nc.vector.memset(ones_row, 1.0)
            ones_bf = small.tile([1, 128], BF16)
            nc.vector.memset(ones_bf, 1.0)

            # ---------------- tiny projections on PE ----------------
            # v = W.T @ [a1|a2] duplicated into both partition halves so
            # upper-quadrant s1 matmuls have a local rhs.
            v_ps = psumA.tile([128, 2], F32, tag="ps", name="v_ps")
            nc.tensor.matmul(v_ps[0:64, :], lhsT=W_lo, rhs=a_lo, start=True, stop=True)
            nc.tensor.matmul(v_ps[64:128, :], lhsT=W_hi, rhs=a_hi, start=True, stop=True)
            v_r = small.tile([128, 2], F32R)
            nc.scalar.copy(out=v_r, in_=v_ps)

            # c = [b.a1, b.a2] ; c12 = c1+c2 broadcast down 128 partitions
            c_ps = psumA.tile([1, 2], F32, tag="ps", name="c_ps")
            nc.tensor.matmul(c_ps, lhsT=b_lo, rhs=a_lo, start=True, stop=True)
            c_sb = small.tile([1, 2], F32)
            nc.scalar.copy(out=c_sb, in_=c_ps)
            cb_ps = psumA.tile([128, 2], F32, tag="ps", name="cb_ps")
            nc.tensor.matmul(cb_ps, lhsT=ones_row, rhs=c_sb, start=True, stop=True)
            c12 = small.tile([128, 1], F32)
            nc.vector.tensor_reduce(out=c12, in_=cb_ps, axis=AX_X, op=ADD)
            c12s = small.tile([128, 1], F32)
            nc.vector.tensor_scalar(
                out=c12s, in0=c12, scalar1=0.01, scalar2=None, op0=MUL
            )

            # ---------------- p2 of own block -> G2row, G2b ----------------
            G2b = persist.tile([128, RB], BF16)
            g2rows = []
            for h in range(2):
                p2r_ps = psumA.tile([1, 512], F32, tag="ps", name="p2r_ps")
                nc.tensor.matmul(
                    p2r_ps,
                    lhsT=v_r[0:64, 1:2],
                    rhs=xq[0:64, h * 512 : (h + 1) * 512],
                    start=True,
                    stop=True,
                )
                g2row = small.tile([1, 512], BF16, tag=f"g2row{h}", name=f"g2row{h}")
                nc.scalar.activation(out=g2row, in_=p2r_ps, func=EXP, scale=-0.99)
                g2rows.append(g2row)
                gb_ps = psumA.tile([128, 512], F32, tag="ps", name="gb_ps")
                nc.tensor.matmul(
                    gb_ps, lhsT=ones_bf, rhs=g2row, start=True, stop=True
                )
                nc.scalar.copy(out=G2b[:, h * 512 : (h + 1) * 512], in_=gb_ps)

            # ---------------- s1 on PE + exps on ACT ----------------
            # s1 tile t: lhsT = xq[quadrant cols of t] [64,128], rhs = v1
            # -> psum [128, 1]; 8 tiles batch into one [128, 8] psum, then
            # ACT exps read the psum directly.
            E1c = small.tile([128, NT], F32)
            F1c = small.tile([128, NT], F32)
            nF1c = small.tile([128, NT], F32)
            # F1 interleaved with zeros (bf16) so TF matmuls have free=2
            F1cb2 = small.tile([128, NT, 2], BF16)
            nc.vector.memset(F1cb2, 0.0)
            for c in CHUNKS:
                # compute both p1,p2 columns per tile (ISA needs even free);
                # only col 0 (p1) is consumed.
                s1ps = psumS.tile([128, 8, 2], F32, tag="s1", name=f"s1ps{c}")
                for k in range(8):
                    t = 8 * c + k
                    if t < 32:
                        lhsT = xq[0:64, 128 * t : 128 * (t + 1)]
                        rhs = v_r[0:64, 0:2]
                    else:
                        lhsT = xq[64:128, 128 * (t - 32) : 128 * (t - 31)]
                        rhs = v_r[64:128, 0:2]
                    nc.tensor.matmul(
                        s1ps[:, k, :],
                        lhsT=lhsT,
                        rhs=rhs,
                        start=True,
                        stop=True,
                        skip_group_check=True,
                    )
                cs = slice(8 * c, 8 * (c + 1))
                p1col = s1ps[:, :, 0:1]
                nc.scalar.activation(
                    out=E1c[:, cs], in_=p1col, func=EXP, bias=c12, scale=1.0
                )
                nc.scalar.activation(
                    out=F1c[:, cs], in_=p1col, func=EXP, bias=c12s, scale=0.01
                )
                nc.scalar.copy(out=F1cb2[:, cs, 0:1], in_=F1c[:, cs])
                nc.vector.tensor_scalar(
                    out=nF1c[:, cs], in0=F1c[:, cs], scalar1=-1.0, scalar2=None,
                    op0=MUL,
                )

            # ---------------- main loop ----------------
            acc0 = psumB.tile([128, 512], F32, tag="acc0", name="acc0")
            acc1 = psumB.tile([128, 512], F32, tag="acc1", name="acc1")
            accs = [acc0, acc1]
            tf_ps = psumB.tile([128, 2], F32, tag="tf", name="tf_ps")
            act_jts = [jt for n, jt in enumerate(JORDER) if EPAT[n % 4] == "A"]
            first_act, last_act = act_jts[0], act_jts[-1]

            for n, jt in enumerate(JORDER):
                eng = EPAT[n % 4]
                e_t = epool.tile([128, RB], BF16, tag="e", name="e_t")
                if eng == "V":
                    nc.vector.tensor_scalar(
                        out=e_t, in0=G2b,
                        scalar1=F1c[:, jt : jt + 1], scalar2=E1c[:, jt : jt + 1],
                        op0=MUL, op1=MAX,
                    )
                elif eng == "P":
                    nc.gpsimd.tensor_scalar(
                        out=e_t, in0=G2b,
                        scalar1=F1c[:, jt : jt + 1], scalar2=E1c[:, jt : jt + 1],
                        op0=MUL, op1=MAX,
                    )
                else:  # ACT: relu(E1 - F1*G2b); rank-1 part restored later
                    nc.scalar.activation(
                        out=e_t, in_=G2b, func=RELU,
                        scale=nF1c[:, jt : jt + 1], bias=E1c[:, jt : jt + 1],
                    )
                lhsT = x_bf[:, jt, 0:DE]
                for h in range(2):
                    nc.tensor.matmul(
                        accs[h][0:DE, :],
                        lhsT=lhsT,
                        rhs=e_t[:, h * 512 : (h + 1) * 512],
                        start=(n == 0),
                        stop=False,
                    )
                if eng == "A":
                    nc.tensor.matmul(
                        tf_ps[0:DE, :],
                        lhsT=lhsT,
                        rhs=F1cb2[:, jt, :],
                        start=(jt == first_act),
                        stop=(jt == last_act),
                    )

            # rank-1 completion: accs[h] += TF_act[d] * G2row[i]
            tf_sb = small.tile([DE, 1], F32)
            nc.scalar.copy(out=tf_sb, in_=tf_ps[0:DE, 0:1])
            tfT_ps = psumA.tile([1, DE], F32, tag="ps", name="tfT_ps")
            nc.tensor.transpose(tfT_ps, tf_sb, ident[:DE, :DE])
            tf_row = small.tile([1, DE], BF16)
            nc.scalar.copy(out=tf_row, in_=tfT_ps)
            for h in range(2):
                nc.tensor.matmul(
                    accs[h][0:DE, :],
                    lhsT=tf_row,
                    rhs=g2rows[h],
                    start=False,
                    stop=True,
                )

            # ---------------- epilogue: normalize + store ----------------
            outT = small.tile([DE, RB], F32)
            for h in range(2):
                nc.scalar.copy(
                    out=outT[:, h * 512 : (h + 1) * 512],
                    in_=accs[h][0:DE, :],
                )
            out_flat = small.tile([128, BT * D], F32)
            out_sb = out_flat.rearrange("p (t d) -> p t d", t=BT)
            for t in range(BT):
                tp2 = psumA.tile([128, DE], F32, tag="ps", name="tp2")
                nc.tensor.transpose(
                    tp2, outT[:, t * 128 : (t + 1) * 128], ident[:DE, :DE]
                )
                rcol = opool.tile([128, 1], F32, tag="rcol", name="rcol")
                nc.vector.reciprocal(rcol, tp2[:, D : D + 1])
                nc.vector.tensor_scalar(
                    out=out_sb[:, t, :],
                    in0=tp2[:, 0:D],
                    scalar1=rcol,
                    scalar2=None,
                    op0=MUL,
                )
            nc.sync.dma_start(out_d[:, :], out_flat)

    nc.finalize()
    return nc


def _execute(inputs: dict, trace: bool = False):
    x = np.ascontiguousarray(np.asarray(inputs["x"], dtype=np.float32))
    W = np.ascontiguousarray(np.asarray(inputs["W"], dtype=np.float32))
    b = np.asarray(inputs["b"], dtype=np.float32).reshape(D)
    a = np.asarray(inputs["a"], dtype=np.float32).reshape(2 * D)
    assert x.shape == (N, D) and W.shape == (D, D)

    pack0 = np.zeros((128, PKW), np.float32)
    for half in (slice(0, 64), slice(64, 128)):
        pack0[half, 0:D] = W
        pack0[half, D] = b
        pack0[half, D + 1] = a[:D]
        pack0[half, D + 2] = a[D:]
    pack0[:, D + 3 : D + 3 + 128] = np.eye(128, dtype=np.float32)

    xe = np.concatenate([x, np.ones((N, 1), np.float32)], axis=1)  # [N, 65]

    nc = build_bass()
    in_maps = []
    for c in range(NCORES):
        # rotate j-order so this core's own block is j 0..1023
        perm = np.concatenate(
            [np.arange(c * RB, (c + 1) * RB), np.arange(0, c * RB),
             np.arange((c + 1) * RB, N)]
        )
        xp = x[perm]            # [N, D] rotated
        xT = xp.T               # [D, N]
        xq = np.concatenate([xT[:, 0:4096], xT[:, 4096:8192]], axis=0)
        xq = np.ascontiguousarray(xq)  # [128, 4096]
        xbf = np.ascontiguousarray(
            xe[perm].reshape(NT, 128, DE)
            .transpose(1, 0, 2)
            .reshape(128, NT * DE)
            .astype(ml_dtypes.bfloat16)
        )
        in_maps.append({"xq": xq, "xbf": xbf, "pack": pack0})
    res = run_bass_kernel_spmd(
        nc, in_maps, core_ids=list(range(NCORES)), trace=trace
    )
    # un-permute each core's output: (p, t*D+d) -> (t*128+p, d)
    outs = []
    for r in res.results:
        o = r["out"].reshape(128, BT, D).transpose(1, 0, 2).reshape(RB, D)
        outs.append(o)
    out = np.ascontiguousarray(np.concatenate(outs, axis=0))
    return out, res


def kernel(x, W, b, a):
    out, _ = _execute({"x": x, "W": W, "b": b, "a": a})
    return out


# revision 10
# speedup vs baseline: 3.9647x; 3.9647x over previous
"""GAT layer (N=8192, D=64) as a Bass/Tile kernel on 8 TRN2 NeuronCores.

Math (reference):
    h  = x @ W.T + b
    s1 = h @ a1 ; s2 = h @ a2                    # [N] each
    score[i,j] = s2[i] + s1[j]
    att = softmax_j(leaky_relu(score))
    out = att @ x

Reformulation:
    Fold the linear layer:  v = W.T @ [a1|a2], c_k = b.a_k
      p1 = x @ v1 ; p2 = x @ v2 ; s1 = p1 + c1 ; s2 = p2 + c2
    Softmax rows are shift invariant, so subtract p2[i] from row i. With
    per-j scalars E1 = exp(sh1), F1 = exp(0.01*sh1) (sh1 = p1 + c1 + c2)
    and the broadcast tile G2b[j,i] = exp(-0.99*p2[i]):
      e[j,i] = max( G2b[j,i] * F1[j],  E1[j] )
    The final matmul (ones-column appended to x for the softmax
    denominator) accumulates over j in PSUM:
      outT[0:65, i] += x_ext[j,:].T @ e[j,i] ; Z[i] = outT[64,i]

Main optimization vs the previous version: e-tiles are produced on THREE
engines concurrently so the PE never starves and can ramp to its max
p-state (2.4 GHz), which halves matmul cost:
  - DVE:  e = max(G2b*F1, E1)          (tensor_scalar, 2x bf16 mode)
  - ACT:  r = relu(E1 - F1*G2b)        (activation Relu, AP scale/bias)
          using max(A,B) = A + relu(B-A); the missing rank-1 part
          A = F1[j]*G2[i] is restored by a single K=1 matmul at the end:
          acc += TF_act[d] * G2row[i],  TF_act[d] = sum_{j in ACT} x~[j,d]F1[j]
          (TF_act accumulates via one 1-column matmul per ACT tile).
  - POOL: same op as DVE (gpsimd tensor_scalar).
The s1 projection runs on PE as 64 tiny quadrant matmuls against xT
(fp32r), with E1/F1 exps reading the PSUM result directly on ACT, so
DVE/POOL/ACT stay dedicated to e-tile production.

Sharding: each core owns N/8 = 1024 query rows i. The host rotates the
j-order per core so the core's own block is always j 0..1023 (all cores
run the identical program; softmax sums are order-invariant).
"""

import sys
import types

import ml_dtypes
import numpy as np

import concourse.bacc as bacc
import concourse.bass as bass
import concourse.mybir as mybir
import concourse.tile as tile
from concourse.bass_utils import run_bass_kernel_spmd


def _install_ntff_hook_shim():
    """The agent image's ``antenv`` lacks ``axon_hooks``; provide it so
    ``run_bass_kernel_spmd(trace=True)`` can capture NTFF profiles."""
    if "antenv.axon_hooks" in sys.modules:
        return
    try:
        from trn_agent_boot.trn_boot import _ntff_profile_via_ctypes

        hook = _ntff_profile_via_ctypes("/opt/axon/libaxon_pjrt.so")
        mod = types.ModuleType("antenv.axon_hooks")
        mod._hook = hook
        mod.get_axon_ntff_profile_hook = lambda: mod._hook
        mod.set_axon_ntff_profile_hook = lambda h: setattr(mod, "_hook", h)
        sys.modules["antenv.axon_hooks"] = mod
    except Exception:
        pass


_install_ntff_hook_shim()

N, D = 8192, 64
NCORES = 8
RB = N // NCORES          # rows (i) per core = 1024
NT = N // 128             # j tiles of 128 = 64
BT = RB // 128            # i tiles per core = 8
DE = D + 1                # x extended with ones column = 65
F32 = mybir.dt.float32
F32R = mybir.dt.float32r
BF16 = mybir.dt.bfloat16
EXP = mybir.ActivationFunctionType.Exp
RELU = mybir.ActivationFunctionType.Relu
ADD = mybir.AluOpType.add
MUL = mybir.AluOpType.mult
MAX = mybir.AluOpType.max
AX_X = mybir.AxisListType.X
PKW = D + 3 + 128  # packed small-input width (W | b | a1 | a2 | ident)

# j-tile processing order: follow the xq DMA slabs (slab k covers lower
# tiles 8k..8k+7 and upper tiles 32+8k..32+8k+7).
CHUNKS = [0, 4, 1, 5, 2, 6, 3, 7]        # s1 chunk order (chunk c = tiles 8c..8c+7)
JORDER = [t for c in CHUNKS for t in range(8 * c, 8 * c + 8)]
# Engine per position in JORDER: DVE 44 tiles, ACT 20.  gpsimd is kept
# OFF the e-stream: its software tensor_scalar runs ~10x below roofline
# for this op on HW and it shares SBUF ports with DVE, dragging
# concurrent DVE ops down with it.  ACT positions sit in the first 56 so
# the TF accumulation closes early and the rank-1 completion overlaps
# the tail of the main loop.
_APOS = set()
for _i in range(20):
    _APOS.add(1 + (_i * 54) // 20)
EPAT = ["A" if n in _APOS else "V" for n in range(64)]


def build_bass() -> bass.Bass:
    nc = bacc.Bacc(None)
    # xq: x.T quadrant-packed, fp32: partitions 0:64 = d, cols j 0..4095;
    # partitions 64:128 = d, cols j 4096..8191 (j in the per-core rotated
    # order; own block is always j 0..1023).
    xq_d = nc.declare_dram_parameter("xq", [128, 4096], F32R, isOutput=False)
    # xbf: bf16 (x | ones) partition-major: [128, t, de]
    xbf_d = nc.declare_dram_parameter("xbf", [128, NT * DE], BF16, isOutput=False)
    pk_d = nc.declare_dram_parameter("pack", [128, PKW], F32, isOutput=False)
    out_d = nc.declare_dram_parameter("out", [128, BT * D], F32, isOutput=True)

    with tile.TileContext(nc) as tc:
        with (
            tc.tile_pool(name="persist", bufs=1) as persist,
            tc.tile_pool(name="small", bufs=1) as small,
            tc.tile_pool(name="epool", bufs=10) as epool,
            tc.tile_pool(name="opool", bufs=2) as opool,
            tc.tile_pool(name="psumA", bufs=3, space="PSUM") as psumA,
            tc.tile_pool(name="psumS", bufs=2, space="PSUM") as psumS,
            tc.tile_pool(name="psumB", bufs=1, space="PSUM") as psumB,
        ):
            # ---------------- DMAs (SP queue, ordered) ----------------
            pk = small.tile([128, PKW], F32)
            nc.sync.dma_start(pk, pk_d[:, :])
            W_lo = pk[0:D, 0:D]
            b_lo = pk[0:D, D : D + 1]
            a_lo = pk[0:D, D + 1 : D + 3]
            W_hi = pk[64:128, 0:D]
            a_hi = pk[64:128, D + 1 : D + 3]
            ident = pk[:, D + 3 : D + 3 + 128]

            xq = persist.tile([128, 4096], F32R)
            xbf_flat = persist.tile([128, NT * DE], BF16)
            x_bf = xbf_flat.rearrange("p (t d) -> p t d", t=NT)
            # slab k of xq covers j-tiles 8k..8k+7 (lower) and 32+8k..39+8k
            # (upper); xbf slab m covers tiles 16m..16m+15.  Issue in the
            # order the pipeline consumes them.
            nc.sync.dma_start(xq[:, 0:1024], xq_d[:, 0:1024])
            nc.sync.dma_start(xbf_flat[:, 0 : 16 * DE], xbf_d[:, 0 : 16 * DE])
            nc.sync.dma_start(
                xbf_flat[:, 32 * DE : 48 * DE], xbf_d[:, 32 * DE : 48 * DE]
            )
            nc.sync.dma_start(xq[:, 1024:2048], xq_d[:, 1024:2048])
            nc.sync.dma_start(xbf_flat[:, 16 * DE : 32 * DE], xbf_d[:, 16 * DE : 32 * DE])
            nc.sync.dma_start(xq[:, 2048:3072], xq_d[:, 2048:3072])
            nc.sync.dma_start(
                xbf_flat[:, 48 * DE : 64 * DE], xbf_d[:, 48 * DE : 64 * DE]
            )
            nc.sync.dma_start(xq[:, 3072:4096], xq_d[:, 3072:4096])

            ones_row = small.tile([1, 128], F32)
            nc.vector.memset(ones_row, 1.0)
            ones_bf = small.tile([1, 128], BF16)
            nc.vector.memset(ones_bf, 1.0)

            # ---------------- tiny projections on PE ----------------
            # v = W.T @ [a1|a2] duplicated into both partition halves so
            # upper-quadrant s1 matmuls have a local rhs.
            v_ps = psumA.tile([128, 2], F32, tag="ps", name="v_ps")
            nc.tensor.matmul(v_ps[0:64, :], lhsT=W_lo, rhs=a_lo, start=True, stop=True)
            nc.tensor.matmul(v_ps[64:128, :], lhsT=W_hi, rhs=a_hi, start=True, stop=True)
            v_r = small.tile([128, 2], F32R)
            nc.scalar.copy(out=v_r, in_=v_ps)

            # c = [b.a1, b.a2] ; c12 = c1+c2 broadcast down 128 partitions
            c_ps = psumA.tile([1, 2], F32, tag="ps", name="c_ps")
            nc.tensor.matmul(c_ps, lhsT=b_lo, rhs=a_lo, start=True, stop=True)
            c_sb = small.tile([1, 2], F32)
            nc.scalar.copy(out=c_sb, in_=c_ps)
            cb_ps = psumA.tile([128, 2], F32, tag="ps", name="cb_ps")
            nc.tensor.matmul(cb_ps, lhsT=ones_row, rhs=c_sb, start=True, stop=True)
            c12 = small.tile([128, 1], F32)
            nc.vector.tensor_reduce(out=c12, in_=cb_ps, axis=AX_X, op=ADD)
            c12s = small.tile([128, 1], F32)
            nc.vector.tensor_scalar(
                out=c12s, in0=c12, scalar1=0.01, scalar2=None, op0=MUL
            )

            # ---------------- p2 of own block -> G2row, G2b ----------------
            G2b = persist.tile([128, RB], BF16)
            g2rows = []
            for h in range(2):
                p2r_ps = psumA.tile([1, 512], F32, tag="ps", name="p2r_ps")
                nc.tensor.matmul(
                    p2r_ps,
                    lhsT=v_r[0:64, 1:2],
                    rhs=xq[0:64, h * 512 : (h + 1) * 512],
                    start=True,
                    stop=True,
                )
                g2row = small.tile([1, 512], BF16, tag=f"g2row{h}", name=f"g2row{h}")
                nc.scalar.activation(out=g2row, in_=p2r_ps, func=EXP, scale=-0.99)
                g2rows.append(g2row)
                gb_ps = psumA.tile([128, 512], F32, tag="ps", name="gb_ps")
                nc.tensor.matmul(
                    gb_ps, lhsT=ones_bf, rhs=g2row, start=True, stop=True
                )
                nc.scalar.copy(out=G2b[:, h * 512 : (h + 1) * 512], in_=gb_ps)

            # ---------------- s1 on PE + exps on ACT ----------------
            # s1 tile t: lhsT = xq[quadrant cols of t] [64,128], rhs = v1
            # -> psum [128, 1]; 8 tiles batch into one [128, 8] psum, then
            # ACT exps read the psum directly.
            E1c = small.tile([128, NT], F32)
            F1c = small.tile([128, NT], F32)
            nF1c = small.tile([128, NT], F32)
            # F1 interleaved with zeros (bf16) so TF matmuls have free=2
            F1cb2 = small.tile([128, NT, 2], BF16)
            nc.vector.memset(F1cb2, 0.0)
            for c in CHUNKS:
                # compute both p1,p2 columns per tile (ISA needs even free);
                # only col 0 (p1) is consumed.
                s1ps = psumS.tile([128, 8, 2], F32, tag="s1", name=f"s1ps{c}")
                for k in range(8):
                    t = 8 * c + k
                    if t < 32:
                        lhsT = xq[0:64, 128 * t : 128 * (t + 1)]
                        rhs = v_r[0:64, 0:2]
                    else:
                        lhsT = xq[64:128, 128 * (t - 32) : 128 * (t - 31)]
                        rhs = v_r[64:128, 0:2]
                    nc.tensor.matmul(
                        s1ps[:, k, :],
                        lhsT=lhsT,
                        rhs=rhs,
                        start=True,
                        stop=True,
                        skip_group_check=True,
                    )
                cs = slice(8 * c, 8 * (c + 1))
                p1col = s1ps[:, :, 0:1]
                nc.scalar.activation(
                    out=E1c[:, cs], in_=p1col, func=EXP, bias=c12, scale=1.0
                )
                nc.scalar.activation(
                    out=F1c[:, cs], in_=p1col, func=EXP, bias=c12s, scale=0.01
                )
                nc.scalar.copy(out=F1cb2[:, cs, 0:1], in_=F1c[:, cs])
                nc.vector.tensor_scalar(
                    out=nF1c[:, cs], in0=F1c[:, cs], scalar1=-1.0, scalar2=None,
                    op0=MUL,
                )

            # ---------------- main loop ----------------
            acc0 = psumB.tile([128, 512], F32, tag="acc0", name="acc0")
            acc1 = psumB.tile([128, 512], F32, tag="acc1", name="acc1")
            accs = [acc0, acc1]
            tf_ps = psumB.tile([128, 2], F32, tag="tf", name="tf_ps")
            act_jts = [jt for n, jt in enumerate(JORDER) if EPAT[n] == "A"]
            first_act, last_act = act_jts[0], act_jts[-1]

            for n, jt in enumerate(JORDER):
                eng = EPAT[n]
                e_t = epool.tile([128, RB], BF16, tag="e", name="e_t")
                if eng == "V":
                    nc.vector.tensor_scalar(
                        out=e_t, in0=G2b,
                        scalar1=F1c[:, jt : jt + 1], scalar2=E1c[:, jt : jt + 1],
                        op0=MUL, op1=MAX,
                    )
                else:  # ACT: relu(E1 - F1*G2b); rank-1 part restored later
                    nc.scalar.activation(
                        out=e_t, in_=G2b, func=RELU,
                        scale=nF1c[:, jt : jt + 1], bias=E1c[:, jt : jt + 1],
                    )
                lhsT = x_bf[:, jt, 0:DE]
                for h in range(2):
                    nc.tensor.matmul(
                        accs[h][0:DE, :],
                        lhsT=lhsT,
                        rhs=e_t[:, h * 512 : (h + 1) * 512],
                        start=(n == 0),
                        stop=False,
                    )
                if eng == "A":
                    nc.tensor.matmul(
                        tf_ps[0:DE, :],
                        lhsT=lhsT,
                        rhs=F1cb2[:, jt, :],
                        start=(jt == first_act),
                        stop=(jt == last_act),
                    )

            # rank-1 completion: accs[h] += TF_act[d] * G2row[i]
            tf_sb = small.tile([DE, 1], F32)
            nc.scalar.copy(out=tf_sb, in_=tf_ps[0:DE, 0:1])
            tfT_ps = psumA.tile([1, DE], F32, tag="ps", name="tfT_ps")
            nc.tensor.transpose(tfT_ps, tf_sb, ident[:DE, :DE])
            tf_row = small.tile([1, DE], BF16)
            nc.scalar.copy(out=tf_row, in_=tfT_ps)
            for h in range(2):
                nc.tensor.matmul(
                    accs[h][0:DE, :],
                    lhsT=tf_row,
                    rhs=g2rows[h],
                    start=False,
                    stop=True,
                )

            # ---------------- epilogue: normalize + store ----------------
            outT = small.tile([DE, RB], F32)
            for h in range(2):
                nc.scalar.copy(
                    out=outT[:, h * 512 : (h + 1) * 512],
                    in_=accs[h][0:DE, :],
                )
            out_flat = small.tile([128, BT * D], F32)
            out_sb = out_flat.rearrange("p (t d) -> p t d", t=BT)
            for t in range(BT):
                tp2 = psumA.tile([128, DE], F32, tag="ps", name="tp2")
                nc.tensor.transpose(
                    tp2, outT[:, t * 128 : (t + 1) * 128], ident[:DE, :DE]
                )
                rcol = opool.tile([128, 1], F32, tag="rcol", name="rcol")
                nc.vector.reciprocal(rcol, tp2[:, D : D + 1])
                nc.vector.tensor_scalar(
                    out=out_sb[:, t, :],
                    in0=tp2[:, 0:D],
                    scalar1=rcol,
                    scalar2=None,
                    op0=MUL,
                )
            nc.sync.dma_start(out_d[:, :], out_flat)

    nc.finalize()
    return nc


def _execute(inputs: dict, trace: bool = False):
    x = np.ascontiguousarray(np.asarray(inputs["x"], dtype=np.float32))
    W = np.ascontiguousarray(np.asarray(inputs["W"], dtype=np.float32))
    b = np.asarray(inputs["b"], dtype=np.float32).reshape(D)
    a = np.asarray(inputs["a"], dtype=np.float32).reshape(2 * D)
    assert x.shape == (N, D) and W.shape == (D, D)

    pack0 = np.zeros((128, PKW), np.float32)
    for half in (slice(0, 64), slice(64, 128)):
        pack0[half, 0:D] = W
        pack0[half, D] = b
        pack0[half, D + 1] = a[:D]
        pack0[half, D + 2] = a[D:]
    pack0[:, D + 3 : D + 3 + 128] = np.eye(128, dtype=np.float32)

    xe = np.concatenate([x, np.ones((N, 1), np.float32)], axis=1)  # [N, 65]

    nc = build_bass()
    in_maps = []
    for c in range(NCORES):
        # rotate j-order so this core's own block is j 0..1023
        perm = np.concatenate(
            [np.arange(c * RB, (c + 1) * RB), np.arange(0, c * RB),
             np.arange((c + 1) * RB, N)]
        )
        xp = x[perm]            # [N, D] rotated
        xT = xp.T               # [D, N]
        xq = np.concatenate([xT[:, 0:4096], xT[:, 4096:8192]], axis=0)
        xq = np.ascontiguousarray(xq)  # [128, 4096]
        xbf = np.ascontiguousarray(
            xe[perm].reshape(NT, 128, DE)
            .transpose(1, 0, 2)
            .reshape(128, NT * DE)
            .astype(ml_dtypes.bfloat16)
        )
        in_maps.append({"xq": xq, "xbf": xbf, "pack": pack0})
    res = run_bass_kernel_spmd(
        nc, in_maps, core_ids=list(range(NCORES)), trace=trace
    )
    # un-permute each core's output: (p, t*D+d) -> (t*128+p, d)
    outs = []
    for r in res.results:
        o = r["out"].reshape(128, BT, D).transpose(1, 0, 2).reshape(RB, D)
        outs.append(o)
    out = np.ascontiguousarray(np.concatenate(outs, axis=0))
    return out, res


def kernel(x, W, b, a):
    out, _ = _execute({"x": x, "W": W, "b": b, "a": a})
    return out
